# revision 39
# baseline (speedup 1.0000x reference)
"""Trainium2 Bass kernel for nn_AdvResNet (dense_mlp, 8 NeuronCores) — fp8.

Reference math (adv=1 path, the one setup_inputs produces):
    beta_norm[n] = sum_k |beta[k, n]|                         # [1024]
    one[n]      = 4096 * sum_h W2[n, h] + bias2[n]            # [1024]
    out[b, n]   = (x @ beta)[b, n] + bias_lin[n]
                  - 0.1 * y[b, n] * beta_norm[n] + one[n]

The weight-derived constants (beta_norm, one) are folded on the host into
per-n scale/bias vectors (exact f32 — like BN folding), so the device does
only the batch-dependent work: the [4096,2048]x[2048,1024] matmul and the
scale*y+bias elementwise term.  Everything streams as fp8 e4m3, matmuls
run DoubleRow (2 contraction rows/cycle), accumulation is f32 in PSUM,
output stores bf16.  Numerics: output is dominated by one[n] ~ 8192, so
the 2e-2 norm rel-err gate leaves ~1 decimal order of margin at the
measured ~1.8e-3 (bf16 store quantization dominates).

Distribution: 2 (n-halves) x 4 (batch-quarters) grid, zero collectives.
Core c = (h=c%2, g=c//2).

Per-core DMA in: xq 2MB + bq 1MB + yq 0.5MB + aux = 3.5MB on two HW rings
(sync+scalar); out 1MB bf16.  PE: 64 DoubleRow matmuls (8 PSUM groups x
8 k-passes) ~= 13.8us issue time at the 2.4GHz warm clock — the per-core
fp8 floor.  The HAM clock gate runs the PE at 1.2GHz until ~3.4us of
sustained activity; warm-up matmuls on memset data pay that ramp while
the first input DMAs are in flight.  Queue throughput scales with
per-partition packet size (~4KB best); transfers are ordered by
consumption time with the first-needed pieces leading both rings.
Phase 2 runs each PSUM group's last 4 k-passes back-to-back so the 8
group stops stagger ~870ns apart and the DVE add + bf16 store for each
group chase its stop instead of serializing after the stream.
"""

import os
import sys

sys.path.insert(0, "/opt/trn_rl_repo")
os.environ.setdefault("NEURON_RT_RESET_CORES", "1")

import ml_dtypes
import numpy as np

import concourse.bass as bass  # noqa: F401
import concourse.tile as tile
from concourse import bacc, mybir
from concourse.bass_utils import run_bass_kernel_spmd

B, NIN, NHID, NOUT = 4096, 2048, 4096, 1024
NC = 8
PN, PB = 2, 4  # core grid: n-halves x batch-quarters
NH = NOUT // PN  # 512 n per core
BSH = B // PB  # 1024 batch rows per core
NT = NH // 128  # 4 n-tiles per core
KP = NIN // 256  # 8 k-passes (DoubleRow contracts 256 per pass)
XC = 4  # xq chunks (2 k-passes each)
BC = 2  # bq chunks (4 k-passes each)
NWARM = 6
EPS = 0.1
F32 = mybir.dt.float32
F8 = mybir.dt.float8e4
BF16 = mybir.dt.bfloat16
DR = mybir.MatmulPerfMode.DoubleRow
NPF8 = ml_dtypes.float8_e4m3
NPBF16 = ml_dtypes.bfloat16

_CACHE = {}


def build_bass():
    nc = bacc.Bacc("TRN2", target_bir_lowering=False, debug=False, num_devices=NC)

    # DRAM params (per core). xq/bq are DoubleRow pair-packed:
    # [chunk][128 part][pass-in-chunk][2][free] with k = (2p+i)*128 + part.
    xq = nc.declare_dram_parameter("xq", [XC, 128, 2, 2, BSH], F8, isOutput=False)
    bq = nc.declare_dram_parameter("bq", [BC, 128, 4, 2, NH], F8, isOutput=False)
    yq = nc.declare_dram_parameter("yq", [128, NT, BSH], F8, isOutput=False)
    # aux f32: cols 0:4 = -EPS*beta_norm (per n-tile), cols 4:8 = one+biases
    auxd = nc.declare_dram_parameter("auxd", [128, 8], F32, isOutput=False)
    out = nc.declare_dram_parameter("out", [NT, 2, 128, 512], BF16, isOutput=True)

    with (
        tile.TileContext(nc) as tc,
        tc.tile_pool(name="xsb", bufs=XC) as xpool,
        tc.tile_pool(name="bsb", bufs=BC) as bpool,
        tc.tile_pool(name="ysb", bufs=1) as ypool,
        tc.tile_pool(name="tsb", bufs=NT) as tpool,
        tc.tile_pool(name="osb", bufs=2 * NT) as opool,
        tc.tile_pool(name="aux", bufs=1) as aux,
        tc.tile_pool(name="psum", bufs=1, space="PSUM") as ppool,
    ):
        ps = [
            [
                ppool.tile([128, 512], F32, name=f"ps{t}_{j}", tag=f"ps{t}_{j}")
                for j in range(2)
            ]
            for t in range(NT)
        ]

        bts = [
            bpool.tile([128, 4, 2, NH], F8, tag="bt", name=f"bt{c}") for c in range(BC)
        ]
        xts = [
            xpool.tile([128, 2, 2, BSH], F8, tag="xt", name=f"xt{c}")
            for c in range(XC)
        ]
        auxt = aux.tile([128, 8], F32)
        yt = ypool.tile([128, NT, BSH], F8, name="yt")

        # Sync ring: first beta pair + x chunks in consumption order, y.
        nc.sync.dma_start(out=bts[0][:, 0:1], in_=bq[0][:, 0:1])
        nc.sync.dma_start(out=xts[0][:, 0:1], in_=xq[0][:, 0:1])
        nc.sync.dma_start(out=xts[0][:, 1:2], in_=xq[0][:, 1:2])
        nc.sync.dma_start(out=xts[1][:, 0:1], in_=xq[1][:, 0:1])
        nc.sync.dma_start(out=xts[1][:, 1:2], in_=xq[1][:, 1:2])
        nc.sync.dma_start(out=xts[2][:], in_=xq[2])
        nc.sync.dma_start(out=yt[:], in_=yq[:])

        # Scalar ring: aux, remaining beta, late x chunk.
        nc.scalar.dma_start(out=auxt[:], in_=auxd[:])
        nc.scalar.dma_start(out=bts[0][:, 1:4], in_=bq[0][:, 1:4])
        nc.scalar.dma_start(out=bts[1][:], in_=bq[1])
        nc.scalar.dma_start(out=xts[3][:], in_=xq[3])

        def blhs(p, t):  # lhsT pass-slice of beta: [128, 2, 128]
            return bts[p // 4][:, p % 4, :, t * 128 : (t + 1) * 128]

        def xrhs(p, j):  # rhs pass-slice of xT: [128, 2, 512]
            return xts[p // 2][:, p % 2, :, j * 512 : (j + 1) * 512]

        # Warm-up matmuls on memset data: pay the HAM cold-clock ramp and
        # first-instruction overheads while the input DMAs are in flight.
        wrm = aux.tile([128, 2, 512], F8)
        nc.gpsimd.memset(wrm[:], 0)
        for _ in range(NWARM):
            nc.tensor.matmul(
                ps[3][1][:],
                lhsT=wrm[:, :, 0:128],
                rhs=wrm[:],
                start=True,
                stop=True,
                perf_mode=DR,
            )

        # t[n,b] = -EPS*beta_norm[n]*y + (one[n]+biases) on ACT, mid-stream
        # (scalar engine is done issuing DMAs by now).
        tts = []
        for t in range(NT):
            tt = tpool.tile([128, BSH], F32, tag="tt", name=f"tt{t}")
            nc.scalar.activation(
                tt[:],
                yt[:, t, :],
                mybir.ActivationFunctionType.Identity,
                bias=auxt[:, 4 + t : 5 + t],
                scale=auxt[:, t : t + 1],
            )
            tts.append(tt)

        # Main stream phase 1 (DMA-paced): k-passes 0..3 for all 8 groups.
        for p in range(KP - 4):
            for t in range(NT):
                for j in range(2):
                    nc.tensor.matmul(
                        ps[t][j][:],
                        lhsT=blhs(p, t),
                        rhs=xrhs(p, j),
                        start=(p == 0),
                        stop=False,
                        perf_mode=DR,
                    )

        # Phase 2 (group-major): each group runs its last 4 k-passes
        # back-to-back, so the 8 stops stagger ~870ns apart and the DVE
        # add + bf16 store for each group chase its stop instead of
        # serializing after the stream (only DVE may touch PSUM).
        obs = []
        for gi, (t, j) in enumerate([(t, j) for t in range(NT) for j in range(2)]):
            for p in range(KP - 4, KP):
                nc.tensor.matmul(
                    ps[t][j][:],
                    lhsT=blhs(p, t),
                    rhs=xrhs(p, j),
                    start=False,
                    stop=(p == KP - 1),
                    perf_mode=DR,
                )
            ob = opool.tile([128, 512], BF16, tag="ob", name=f"ob{t}_{j}")
            seng = nc.sync if gi % 2 == 0 else nc.scalar
            nc.vector.tensor_add(
                ob[:], ps[t][j][:], tts[t][:, j * 512 : (j + 1) * 512]
            )
            seng.dma_start(out=out[t][j], in_=ob[:])
            obs.append(ob)

    nc.compile()
    return nc


def _get_nc():
    if "nc" not in _CACHE:
        _CACHE["nc"] = build_bass()
    return _CACHE["nc"]


def _pack_pairs(a):
    """[K, F] -> [K//256, 128, 2, F] with k = (2p+i)*128 + r."""
    k, f = a.shape
    return np.ascontiguousarray(
        a.reshape(k // 256, 2, 128, f).transpose(0, 2, 1, 3)
    )


def _shard_inputs(x, y, beta, bias_lin, W2, bias2):
    x32 = np.asarray(x, np.float32)
    y32 = np.asarray(y, np.float32)
    b32 = np.asarray(beta, np.float32)
    x8 = x32.astype(NPF8)
    y8 = y32.astype(NPF8)
    b8 = b32.astype(NPF8)

    # Host-folded weight constants (exact f32): scale[n] = -EPS*||beta[:,n]||_1,
    # bias[n] = NHID*sum_h W2[n,h] + bias2[n] + bias_lin[n].
    scale = (-EPS * np.abs(b32).sum(axis=0)).astype(np.float32)
    biasc = (
        float(NHID) * np.asarray(W2, np.float32).sum(axis=1)
        + np.asarray(bias2, np.float32)
        + np.asarray(bias_lin, np.float32)
    ).astype(np.float32)

    aux_h = []
    for h in range(PN):
        a = np.zeros((128, 8), np.float32)
        a[:, 0:4] = scale[h * NH : (h + 1) * NH].reshape(NT, 128).T
        a[:, 4:8] = biasc[h * NH : (h + 1) * NH].reshape(NT, 128).T
        aux_h.append(a)

    # xq per batch-quarter: pair-packed xT
    xq_g = []
    for g in range(PB):
        xT = np.ascontiguousarray(x8[g * BSH : (g + 1) * BSH, :].T)  # [NIN, BSH]
        xq_g.append(_pack_pairs(xT).reshape(XC, 128, 2, 2, BSH))
    # bq per n-half: pair-packed beta[:, nh]
    bq_h = [
        _pack_pairs(np.ascontiguousarray(b8[:, h * NH : (h + 1) * NH])).reshape(
            BC, 128, 4, 2, NH
        )
        for h in range(PN)
    ]
    in_maps = []
    for c in range(NC):
        h, g = c % PN, c // PN
        yT = np.ascontiguousarray(
            y8[g * BSH : (g + 1) * BSH, h * NH : (h + 1) * NH].T
        ).reshape(NT, 128, BSH).transpose(1, 0, 2)
        in_maps.append(
            {
                "xq": xq_g[g],
                "bq": bq_h[h],
                "yq": np.ascontiguousarray(yT),
                "auxd": aux_h[h],
            }
        )
    return in_maps


def run_device(inputs, trace=False, **kw):
    nc = _get_nc()
    in_maps = _shard_inputs(
        inputs["x"], inputs["y"], inputs["beta"], inputs["bias_lin"],
        inputs["W2"], inputs["bias2"],
    )
    res = run_bass_kernel_spmd(nc, in_maps, core_ids=list(range(NC)), trace=trace, **kw)
    full = np.empty((B, NOUT), dtype=np.float32)
    for c in range(NC):
        h, g = c % PN, c // PN
        arr = res.results[c]["out"].astype(np.float32)  # [NT, 2, 128, 512]
        full[g * BSH : (g + 1) * BSH, h * NH : (h + 1) * NH] = (
            arr.transpose(1, 3, 0, 2).reshape(BSH, NH)
        )
    return full, res


def _reference_numpy(x, y, beta, bias_lin, W1, W2, bias1, bias2, adv):
    # Fallback for the adv=0 path (never produced by setup_inputs).
    x = np.asarray(x, np.float32)
    lin = x @ np.asarray(beta, np.float32) + np.asarray(bias_lin, np.float32)
    if adv:
        beta_norm = np.sum(np.abs(np.asarray(beta, np.float32)), axis=0)
        lin = lin - EPS * np.asarray(y, np.float32) * beta_norm
        one = NHID * np.sum(np.asarray(W2, np.float32), axis=1) + np.asarray(
            bias2, np.float32
        )
        one = np.broadcast_to(one, lin.shape)
    else:
        h = np.maximum(
            x @ np.asarray(W1, np.float32).T + np.asarray(bias1, np.float32), 0.0
        )
        one = h @ np.asarray(W2, np.float32).T + np.asarray(bias2, np.float32)
    return (lin + one).astype(np.float32)


def kernel(**inputs) -> np.ndarray:
    adv = int(np.asarray(inputs.get("adv", 1)))
    if adv == 0:
        return _reference_numpy(
            inputs["x"], inputs["y"], inputs["beta"], inputs["bias_lin"],
            inputs["W1"], inputs["W2"], inputs["bias1"], inputs["bias2"], adv,
        )
    full, _ = run_device(inputs)
    return full


# revision 48
# speedup vs baseline: 1.0256x; 1.0256x over previous
"""Trainium2 Bass kernel for nn_AdvResNet (dense_mlp, 8 NeuronCores) — fp8.

Reference math (adv=1 path, the one setup_inputs produces):
    beta_norm[n] = sum_k |beta[k, n]|                         # [1024]
    one[n]      = 4096 * sum_h W2[n, h] + bias2[n]            # [1024]
    out[b, n]   = (x @ beta)[b, n] + bias_lin[n]
                  - 0.1 * y[b, n] * beta_norm[n] + one[n]

The weight-derived constants (beta_norm, one) are folded on the host into
per-n scale/bias vectors (exact f32 — like BN folding), so the device does
only the batch-dependent work: the [4096,2048]x[2048,1024] matmul and the
scale*y+bias elementwise term.  Everything streams as fp8 e4m3, matmuls
run DoubleRow (2 contraction rows/cycle), accumulation is f32 in PSUM,
output stores bf16.  Numerics: output is dominated by one[n] ~ 8192, so
the 2e-2 norm rel-err gate leaves ~1 decimal order of margin at the
measured ~1.8e-3 (bf16 store quantization dominates).

Distribution: 2 (n-halves) x 4 (batch-quarters) grid, zero collectives.
Core c = (h=c%2, g=c//2).

Per-core DMA in: xq 2MB + bq 1MB + yq 0.5MB + aux = 3.5MB on two HW rings
(sync+scalar); out 1MB bf16.  PE: 64 DoubleRow matmuls (8 PSUM groups x
8 k-passes) ~= 13.8us issue time at the 2.4GHz warm clock — the per-core
fp8 floor.  The HAM clock gate runs the PE at 1.2GHz until ~3.4us of
sustained activity; warm-up matmuls on memset data pay that ramp while
the first input DMAs are in flight.  Queue throughput scales with
per-partition packet size (~4KB best); transfers are ordered by
consumption time with the first-needed pieces leading both rings.
Phase 2 runs each PSUM group's last 4 k-passes back-to-back so the 8
group stops stagger ~870ns apart and the DVE add + bf16 store for each
group chase its stop instead of serializing after the stream.
"""

import os
import sys

sys.path.insert(0, "/opt/trn_rl_repo")
os.environ.setdefault("NEURON_RT_RESET_CORES", "1")

import ml_dtypes
import numpy as np

import concourse.bass as bass  # noqa: F401
import concourse.tile as tile
from concourse import bacc, mybir
from concourse.bass_utils import run_bass_kernel_spmd

B, NIN, NHID, NOUT = 4096, 2048, 4096, 1024
NC = 8
PN, PB = 2, 4  # core grid: n-halves x batch-quarters
NH = NOUT // PN  # 512 n per core
BSH = B // PB  # 1024 batch rows per core
NT = NH // 128  # 4 n-tiles per core
KP = NIN // 256  # 8 k-passes (DoubleRow contracts 256 per pass)
XC = 4  # xq chunks (2 k-passes each)
BC = 2  # bq chunks (4 k-passes each)
NWARM = 30  # N=128 warmups, ~107ns cold each: ~3.2us of HAM-warming coverage
EPS = 0.1
F32 = mybir.dt.float32
F8 = mybir.dt.float8e4
BF16 = mybir.dt.bfloat16
DR = mybir.MatmulPerfMode.DoubleRow
NPF8 = ml_dtypes.float8_e4m3
NPBF16 = ml_dtypes.bfloat16

_CACHE = {}


def build_bass():
    nc = bacc.Bacc("TRN2", target_bir_lowering=False, debug=False, num_devices=NC)

    # DRAM params (per core). xq/bq are DoubleRow pair-packed:
    # [chunk][128 part][pass-in-chunk][2][free] with k = (2p+i)*128 + part.
    xq = nc.declare_dram_parameter("xq", [XC, 128, 2, 2, BSH], F8, isOutput=False)
    bq = nc.declare_dram_parameter("bq", [BC, 128, 4, 2, NH], F8, isOutput=False)
    yq = nc.declare_dram_parameter("yq", [128, NT, BSH], F8, isOutput=False)
    # aux f32: cols 0:4 = -EPS*beta_norm (per n-tile), cols 4:8 = one+biases
    auxd = nc.declare_dram_parameter("auxd", [128, 8], F32, isOutput=False)
    out = nc.declare_dram_parameter("out", [NT, 2, 128, 512], BF16, isOutput=True)

    with (
        tile.TileContext(nc) as tc,
        tc.tile_pool(name="xsb", bufs=XC) as xpool,
        tc.tile_pool(name="bsb", bufs=BC) as bpool,
        tc.tile_pool(name="ysb", bufs=1) as ypool,
        tc.tile_pool(name="tsb", bufs=NT) as tpool,
        tc.tile_pool(name="osb", bufs=2 * NT) as opool,
        tc.tile_pool(name="aux", bufs=1) as aux,
        tc.tile_pool(name="psum", bufs=1, space="PSUM") as ppool,
    ):
        ps = [
            [
                ppool.tile([128, 512], F32, name=f"ps{t}_{j}", tag=f"ps{t}_{j}")
                for j in range(2)
            ]
            for t in range(NT)
        ]

        bts = [
            bpool.tile([128, 4, 2, NH], F8, tag="bt", name=f"bt{c}") for c in range(BC)
        ]
        xts = [
            xpool.tile([128, 2, 2, BSH], F8, tag="xt", name=f"xt{c}")
            for c in range(XC)
        ]
        auxt = aux.tile([128, 8], F32)
        yt = ypool.tile([128, NT, BSH], F8, name="yt")

        # Sync ring: all x chunks in consumption order, then y tiles 0-1.
        # The scalar ring leads with the pass-0 beta pair so the two
        # pieces gating the first real matmul transfer in parallel.
        nc.sync.dma_start(out=xts[0][:, 0:1], in_=xq[0][:, 0:1])
        nc.sync.dma_start(out=xts[0][:, 1:2], in_=xq[0][:, 1:2])
        nc.sync.dma_start(out=xts[1][:, 0:1], in_=xq[1][:, 0:1])
        nc.sync.dma_start(out=xts[1][:, 1:2], in_=xq[1][:, 1:2])
        nc.sync.dma_start(out=xts[2][:], in_=xq[2])
        nc.sync.dma_start(out=xts[3][:], in_=xq[3])
        nc.sync.dma_start(out=yt[:, 2:4], in_=yq[:, 2:4])

        # Scalar ring: pass-0 beta, aux, remaining beta split per
        # consumption, y tiles 2-3.
        nc.scalar.dma_start(out=bts[0][:, 0:1], in_=bq[0][:, 0:1])
        nc.scalar.dma_start(out=auxt[:], in_=auxd[:])
        nc.scalar.dma_start(out=bts[0][:, 1:2], in_=bq[0][:, 1:2])
        nc.scalar.dma_start(out=bts[0][:, 2:4], in_=bq[0][:, 2:4])
        nc.scalar.dma_start(out=bts[1][:], in_=bq[1])
        nc.scalar.dma_start(out=yt[:, 0:2], in_=yq[:, 0:2])

        def blhs(p, t):  # lhsT pass-slice of beta: [128, 2, 128]
            return bts[p // 4][:, p % 4, :, t * 128 : (t + 1) * 128]

        def xrhs(p, j):  # rhs pass-slice of xT: [128, 2, 512]
            return xts[p // 2][:, p % 2, :, j * 512 : (j + 1) * 512]

        # Warm-up matmuls on memset data: pay the HAM cold-clock ramp and
        # first-instruction overheads while the input DMAs are in flight.
        # N=128 keeps the memset small (fast unblock off the loop-entry
        # barrier) and makes the warmup->real seam fine-grained.
        wrm = aux.tile([128, 2, 128], F8)
        nc.gpsimd.memset(wrm[:], 0)
        for _ in range(NWARM):
            nc.tensor.matmul(
                ps[0][0][:, 0:128],
                lhsT=wrm[:],
                rhs=wrm[:],
                start=True,
                stop=True,
                perf_mode=DR,
            )

        # t[n,b] = -EPS*beta_norm[n]*y + (one[n]+biases).  Tiles 0-1 on
        # the ACT engine (their y half lands early on the scalar ring);
        # tiles 2-3 on the otherwise-idle GpSimd (SBUF->SBUF tensor_scalar
        # is legal there) so the DVE only ever runs the epilogue adds and
        # the late y half never queues in front of them.
        tts = []
        for t in range(NT):
            tt = tpool.tile([128, BSH], F32, tag="tt", name=f"tt{t}")
            if t < 2:
                nc.scalar.activation(
                    tt[:],
                    yt[:, t, :],
                    mybir.ActivationFunctionType.Identity,
                    bias=auxt[:, 4 + t : 5 + t],
                    scale=auxt[:, t : t + 1],
                )
            else:
                nc.gpsimd.tensor_scalar(
                    tt[:],
                    yt[:, t, :],
                    auxt[:, t : t + 1],
                    auxt[:, 4 + t : 5 + t],
                    op0=mybir.AluOpType.mult,
                    op1=mybir.AluOpType.add,
                )
            tts.append(tt)

        # Main stream phase 1 (DMA-paced): k-passes 0..3 for all 8 groups.
        for p in range(KP - 4):
            for t in range(NT):
                for j in range(2):
                    nc.tensor.matmul(
                        ps[t][j][:],
                        lhsT=blhs(p, t),
                        rhs=xrhs(p, j),
                        start=(p == 0),
                        stop=False,
                        perf_mode=DR,
                    )

        # Phase 2 (group-major): each group runs its last 4 k-passes
        # back-to-back, so the 8 stops stagger ~870ns apart and the DVE
        # add + bf16 store for each group chase its stop instead of
        # serializing after the stream (only DVE may touch PSUM).
        obs = []
        for gi, (t, j) in enumerate([(t, j) for t in range(NT) for j in range(2)]):
            for p in range(KP - 4, KP):
                nc.tensor.matmul(
                    ps[t][j][:],
                    lhsT=blhs(p, t),
                    rhs=xrhs(p, j),
                    start=False,
                    stop=(p == KP - 1),
                    perf_mode=DR,
                )
            ob = opool.tile([128, 512], BF16, tag="ob", name=f"ob{t}_{j}")
            seng = nc.sync if gi % 2 == 0 else nc.scalar
            nc.vector.tensor_add(
                ob[:], ps[t][j][:], tts[t][:, j * 512 : (j + 1) * 512]
            )
            seng.dma_start(out=out[t][j], in_=ob[:])
            obs.append(ob)

    nc.compile()
    return nc


def _get_nc():
    if "nc" not in _CACHE:
        _CACHE["nc"] = build_bass()
    return _CACHE["nc"]


def _pack_pairs(a):
    """[K, F] -> [K//256, 128, 2, F] with k = (2p+i)*128 + r."""
    k, f = a.shape
    return np.ascontiguousarray(
        a.reshape(k // 256, 2, 128, f).transpose(0, 2, 1, 3)
    )


def _shard_inputs(x, y, beta, bias_lin, W2, bias2):
    x32 = np.asarray(x, np.float32)
    y32 = np.asarray(y, np.float32)
    b32 = np.asarray(beta, np.float32)
    x8 = x32.astype(NPF8)
    y8 = y32.astype(NPF8)
    b8 = b32.astype(NPF8)

    # Host-folded weight constants (exact f32): scale[n] = -EPS*||beta[:,n]||_1,
    # bias[n] = NHID*sum_h W2[n,h] + bias2[n] + bias_lin[n].
    scale = (-EPS * np.abs(b32).sum(axis=0)).astype(np.float32)
    biasc = (
        float(NHID) * np.asarray(W2, np.float32).sum(axis=1)
        + np.asarray(bias2, np.float32)
        + np.asarray(bias_lin, np.float32)
    ).astype(np.float32)

    aux_h = []
    for h in range(PN):
        a = np.zeros((128, 8), np.float32)
        a[:, 0:4] = scale[h * NH : (h + 1) * NH].reshape(NT, 128).T
        a[:, 4:8] = biasc[h * NH : (h + 1) * NH].reshape(NT, 128).T
        aux_h.append(a)

    # xq per batch-quarter: pair-packed xT
    xq_g = []
    for g in range(PB):
        xT = np.ascontiguousarray(x8[g * BSH : (g + 1) * BSH, :].T)  # [NIN, BSH]
        xq_g.append(_pack_pairs(xT).reshape(XC, 128, 2, 2, BSH))
    # bq per n-half: pair-packed beta[:, nh]
    bq_h = [
        _pack_pairs(np.ascontiguousarray(b8[:, h * NH : (h + 1) * NH])).reshape(
            BC, 128, 4, 2, NH
        )
        for h in range(PN)
    ]
    in_maps = []
    for c in range(NC):
        h, g = c % PN, c // PN
        yT = np.ascontiguousarray(
            y8[g * BSH : (g + 1) * BSH, h * NH : (h + 1) * NH].T
        ).reshape(NT, 128, BSH).transpose(1, 0, 2)
        in_maps.append(
            {
                "xq": xq_g[g],
                "bq": bq_h[h],
                "yq": np.ascontiguousarray(yT),
                "auxd": aux_h[h],
            }
        )
    return in_maps


def run_device(inputs, trace=False, **kw):
    nc = _get_nc()
    in_maps = _shard_inputs(
        inputs["x"], inputs["y"], inputs["beta"], inputs["bias_lin"],
        inputs["W2"], inputs["bias2"],
    )
    res = run_bass_kernel_spmd(nc, in_maps, core_ids=list(range(NC)), trace=trace, **kw)
    full = np.empty((B, NOUT), dtype=np.float32)
    for c in range(NC):
        h, g = c % PN, c // PN
        arr = res.results[c]["out"].astype(np.float32)  # [NT, 2, 128, 512]
        full[g * BSH : (g + 1) * BSH, h * NH : (h + 1) * NH] = (
            arr.transpose(1, 3, 0, 2).reshape(BSH, NH)
        )
    return full, res


def _reference_numpy(x, y, beta, bias_lin, W1, W2, bias1, bias2, adv):
    # Fallback for the adv=0 path (never produced by setup_inputs).
    x = np.asarray(x, np.float32)
    lin = x @ np.asarray(beta, np.float32) + np.asarray(bias_lin, np.float32)
    if adv:
        beta_norm = np.sum(np.abs(np.asarray(beta, np.float32)), axis=0)
        lin = lin - EPS * np.asarray(y, np.float32) * beta_norm
        one = NHID * np.sum(np.asarray(W2, np.float32), axis=1) + np.asarray(
            bias2, np.float32
        )
        one = np.broadcast_to(one, lin.shape)
    else:
        h = np.maximum(
            x @ np.asarray(W1, np.float32).T + np.asarray(bias1, np.float32), 0.0
        )
        one = h @ np.asarray(W2, np.float32).T + np.asarray(bias2, np.float32)
    return (lin + one).astype(np.float32)


def kernel(**inputs) -> np.ndarray:
    adv = int(np.asarray(inputs.get("adv", 1)))
    if adv == 0:
        return _reference_numpy(
            inputs["x"], inputs["y"], inputs["beta"], inputs["bias_lin"],
            inputs["W1"], inputs["W2"], inputs["bias1"], inputs["bias2"], adv,
        )
    full, _ = run_device(inputs)
    return full
